# revision 5
# baseline (speedup 1.0000x reference)
"""Trainium2 Bass kernel for nn_DiffAttention (GNN message passing).

Math (per edge i: src s_i -> dst n, dst sorted):
  d_i = (h_dst[n] - h_src[s_i]) @ W_fc.T ;  e_i = tanh(d_i @ w_attn)
  alpha = segment_softmax(e, dst);  out[n] = elu(sum_i alpha_i d_i)
Since e in [-1,1], softmax needs no max-subtraction:
  out[n] = elu(p_dst[n] - (sum_i w_i p_src[s_i]) / (sum_i w_i)),
  w_i = exp(tanh(q_dst[n] - q_src[s_i])), p = h @ W_fc.T, q = p @ w_attn.

v2 device strategy (8 cores, SPMD, edge-parallel by dst range):
  - src table [NPAD, 128] f16 rows [p_src(0:64) | 1 | q_src | pad], built
    sharded on PE then AllGather; dst table [NPAD, 66] f16 rows
    [p_dst | _ | q_dst].
  - per window (<=128 consecutive dst nodes, 2048 edge slots grouped as
    4 subtables x 512): 4 dma_gather ops (int16 subtable-local indices)
    fetch all edge rows; window dst rows via one [128,1] indirect gather.
  - qb[p, j] = q_dst(window node j) broadcast via K=1 matmul.
  - per 128-edge tile: one fused DVE op gives qd (accum of onehot*qb);
    batched tanh/exp on ACT for all 16 tiles at once; one fused DVE op
    builds Sw = onehot*w (f16); PE accumulates [sum w*p | sum w] in PSUM.
  - epilogue per window: elu(p_dst - swp/sw) with zero-edge masking.
Host does only index prep (windowing, grouping by src subtable, int16
wrapping, f16 casts) and reassembles per-window outputs into node order.
"""
import sys
sys.path.insert(0, "/opt/trn_rl_repo")
import numpy as np

N = 100000
D = 64
NC = 8
K = 16            # 128-edge tiles per window (4 groups x 4 tiles)
NSUB = 4          # src-node subtables (int16 index range)
SUBR = 25088      # rows per subtable
GCAP = 512        # edge slots per (window, subtable) group
WE = NSUB * GCAP  # 2048 edge slots per window
WIN_NODES = 128
SHARD = 12544     # 98*128 rows built per core
DROWS = 12800     # local dst-table rows (core's window node range, padded)
NPAD = NC * SHARD # 100352 = 4*25088
DUMMY = N         # all-zero row (h padded with zeros)
ROWS = 128        # src table row width (f16): [p(0:64) | 1 | q | pad]
ROWD = 66         # dst table row width (f16): [p(0:64) | _ | q]
MAIN_REPEAT = 1   # test.py overrides for timing


# ---------------------------------------------------------------- host prep
def _partition_edges(dst):
    E = dst.shape[0]
    bounds, e_prev, n_prev = [], 0, 0
    for c in range(1, NC):
        s = (E * c) // NC
        while 0 < s < E and dst[s] == dst[s - 1]:
            s += 1
        node_split = int(dst[s]) if s < E else N
        bounds.append((e_prev, s, n_prev, node_split))
        e_prev, n_prev = s, node_split
    bounds.append((e_prev, E, n_prev, N))
    return bounds


def _build_windows(src, dst, e_lo, e_hi, n_lo, n_hi):
    n_total = n_hi - n_lo
    dl = dst[e_lo:e_hi] - n_lo
    sub = src[e_lo:e_hi] // SUBR
    # per-subtable cumulative edge counts by node
    cums = np.zeros((NSUB, n_total + 1), np.int64)
    for s in range(NSUB):
        cums[s, 1:] = np.cumsum(np.bincount(
            dl[sub == s], minlength=n_total))
    counts = np.bincount(dl, minlength=n_total)
    starts = np.concatenate([[0], np.cumsum(counts)])
    sidxs, dlocs, bases, nns = [], [], [], []
    n = 0
    while n < n_total:
        n_end = min(n + WIN_NODES, n_total)
        while n_end > n and np.any(cums[:, n_end] - cums[:, n] > GCAP):
            n_end -= 1
        assert n_end > n, "node degree exceeds per-subtable window cap"
        e0, e1 = int(starts[n]), int(starts[n_end])
        esub = sub[e0:e1]
        edl = dl[e0:e1] - n
        esrc = src[e_lo + e0:e_lo + e1]
        sidx16 = np.zeros((128, NSUB * 32), np.int16)
        dloc = np.full((128, K), -1.0, np.float32)
        for s in range(NSUB):
            sel = esub == s
            c = int(sel.sum())
            arr = np.zeros(GCAP, np.int16)
            arr[:c] = (esrc[sel] - s * SUBR).astype(np.int16)
            wrapped = arr.reshape(32, 16).T                 # [16, 32]
            sidx16[:, s * 32:(s + 1) * 32] = np.tile(wrapped, (8, 1))
            i = np.arange(c)
            dloc[i % 128, s * 4 + i // 128] = edl[sel].astype(np.float32)
        sidxs.append(sidx16); dlocs.append(dloc)
        bases.append(n_lo + n); nns.append(n_end - n)
        n = n_end
    return np.stack(sidxs), np.stack(dlocs), np.array(bases), np.array(nns)


def _prep(src, dst):
    src = np.asarray(src, np.int64)
    dst = np.asarray(dst, np.int64)
    if np.any(np.diff(dst) < 0):  # tolerate unsorted edges
        order = np.argsort(dst, kind="stable")
        src, dst = src[order], dst[order]
    bounds = _partition_edges(dst)
    per_core = [_build_windows(src, dst, *b) for b in bounds]
    nW = max(p[0].shape[0] for p in per_core)
    cores = []
    for (e_lo, e_hi, n_lo, n_hi), (si, dl, base, nn) in zip(bounds, per_core):
        assert n_hi - n_lo <= DROWS - 128, "core node range exceeds DROWS"
        pad = nW - si.shape[0]
        if pad:
            si = np.concatenate([si, np.zeros((pad, 128, NSUB * 32), np.int16)])
            dl = np.concatenate([dl, np.full((pad, 128, K), -1.0, np.float32)])
            base = np.concatenate([base, np.full(pad, N, np.int64)])
            nn = np.concatenate([nn, np.zeros(pad, np.int64)])
        slot = (base[:, None] - n_lo) + np.arange(WIN_NODES)[None, :]
        slot = np.where(np.arange(WIN_NODES)[None, :] < nn[:, None], slot, 0)
        slot = np.clip(slot, 0, DROWS - 1)
        cores.append(dict(sidx=si, dloc=dl, base=base, nn=nn, n_lo=n_lo,
                          slot_ids=slot.astype(np.int32)))
    return cores, nW


# ---------------------------------------------------------------- device
def _build_program(nW, main_repeat, ablate=""):
    from concourse import bass, bacc, mybir, tile
    f32, f16 = mybir.dt.float32, mybir.dt.float16
    i16, i32 = mybir.dt.int16, mybir.dt.int32

    nc = bacc.Bacc("TRN2", target_bir_lowering=False, debug=False,
                   num_devices=NC, num_swdge_queues=4)
    hs_e = nc.dram_tensor("hs", [D, SHARD], f32, kind="ExternalInput")
    hd_e = nc.dram_tensor("hd", [D, DROWS], f32, kind="ExternalInput")
    wfc_e = nc.dram_tensor("wfc", [D, D], f32, kind="ExternalInput")
    wat_e = nc.dram_tensor("wat", [D, 1], f32, kind="ExternalInput")
    sidx_e = nc.dram_tensor("sidx", [128, nW, NSUB * 32], i16,
                            kind="ExternalInput")
    dloc_e = nc.dram_tensor("dloc", [128, nW, K], f32, kind="ExternalInput")
    nid_e = nc.dram_tensor("nid", [128, nW, 1], i32, kind="ExternalInput")
    res_e = nc.dram_tensor("res", [nW * 128, D], f32, kind="ExternalOutput")

    AL = mybir.AluOpType
    AF = mybir.ActivationFunctionType

    with tile.TileContext(nc) as tc:
        with tc.tile_pool(name="c", bufs=1) as cp, \
             tc.tile_pool(name="sb", bufs=3) as sp, \
             tc.tile_pool(name="dr", bufs=1, space="DRAM") as dp:
            pp = tc.alloc_tile_pool(name="psb", bufs=1, space="PSUM")
            # ---- constants
            ident_d = nc.inline_tensor(np.eye(128, dtype=np.float32),
                                       name="ident_c")
            iota16_d = nc.inline_tensor(
                np.tile(np.arange(128, dtype=np.float16), (128, 1)),
                name="iota16_c")
            ident = cp.tile([128, 128], f32)
            nc.sync.dma_start(out=ident[:], in_=ident_d[:])
            iota16 = cp.tile([128, 128], f16)
            nc.sync.dma_start(out=iota16[:], in_=iota16_d[:])
            ident16_d = nc.inline_tensor(np.eye(128, dtype=np.float16),
                                         name="ident16_c")
            ident16 = cp.tile([128, 128], f16)
            nc.sync.dma_start(out=ident16[:], in_=ident16_d[:])
            ones_row = cp.tile([1, 128], f16)
            nc.vector.memset(ones_row[:], 1.0)
            ones_col = cp.tile([128, 1], f32)
            nc.vector.memset(ones_col[:], 1.0)

            # ---- weight prep: rhsb [64, 66] = [W.T | 0 | W.T @ w_attn]
            wfc = cp.tile([D, D], f32)
            nc.sync.dma_start(out=wfc[:], in_=wfc_e[:])
            wat = cp.tile([D, 1], f32)
            nc.sync.dma_start(out=wat[:], in_=wat_e[:])
            wt_ps = pp.tile([D, D], f32, space="PSUM")
            nc.tensor.transpose(out=wt_ps[:], in_=wfc[:], identity=ident[:D, :D])
            v_ps = pp.tile([D, 1], f32, space="PSUM")
            nc.tensor.matmul(out=v_ps[:], lhsT=wfc[:], rhs=wat[:],
                             start=True, stop=True)
            rhsb = cp.tile([D, ROWD], f32)
            nc.vector.memset(rhsb[:], 0.0)
            nc.vector.tensor_copy(rhsb[:, 0:64], wt_ps[:])
            nc.vector.tensor_copy(rhsb[:, 65:66], v_ps[:])

            # ---- table build: src rows from this core's shard,
            # dst rows local to this core's window node range
            bp = tc.alloc_tile_pool(name="bld", bufs=1)
            src_sh = dp.tile([SHARD, ROWS], f16)
            dst_tbl = dp.tile([DROWS, ROWD], f16)
            pro_ctx = (tc.For_i(0, main_repeat, 1)
                       if ablate == "prologue_repeat" and main_repeat > 1
                       else None)
            if pro_ctx is not None:
                pro_ctx.__enter__()
            hsT = bp.tile([D, SHARD], f32)
            nc.sync.dma_start(out=hsT[:], in_=hs_e[:])
            hdT = bp.tile([D, DROWS], f32)
            nc.sync.dma_start(out=hdT[:], in_=hd_e[:])
            for j in range(SHARD // 128):
                r0 = j * 128
                ps_s = pp.tile([128, ROWD], f32, space="PSUM", tag="bp", bufs=3)
                nc.tensor.matmul(out=ps_s[:], lhsT=hsT[:, r0:r0 + 128],
                                 rhs=rhsb[:], start=True, stop=True)
                tbs = sp.tile([128, ROWS], f16, tag="bo")
                nc.vector.memset(tbs[:], 0.0)
                nc.vector.tensor_copy(tbs[:, 0:ROWD], ps_s[:])
                nc.vector.memset(tbs[:, 64:65], 1.0)
                if j % 2 == 0:
                    nc.sync.dma_start(out=src_sh[r0:r0 + 128, :], in_=tbs[:])
                else:
                    nc.scalar.dma_start(out=src_sh[r0:r0 + 128, :], in_=tbs[:])
            for j in range(DROWS // 128):
                r0 = j * 128
                ps_d = pp.tile([128, ROWD], f32, space="PSUM", tag="bp2", bufs=3)
                nc.tensor.matmul(out=ps_d[:], lhsT=hdT[:, r0:r0 + 128],
                                 rhs=rhsb[:], start=True, stop=True)
                tbd = sp.tile([128, ROWD], f16, tag="bo2")
                nc.vector.tensor_copy(tbd[:], ps_d[:])
                if j % 2 == 0:
                    nc.sync.dma_start(out=dst_tbl[r0:r0 + 128, :], in_=tbd[:])
                else:
                    nc.scalar.dma_start(out=dst_tbl[r0:r0 + 128, :], in_=tbd[:])
            if pro_ctx is not None:
                pro_ctx.__exit__(None, None, None)
                pro_ctx = None
            bp.release()

            pp.release()
            pp2 = tc.alloc_tile_pool(name="psm", bufs=2, space="PSUM")

            # ---- all-gather the tables
            src_tbl = dp.tile([NPAD, ROWS], f16)
            nc.gpsimd.collective_compute(
                "AllGather", mybir.AluOpType.bypass,
                replica_groups=[list(range(NC))],
                ins=[src_sh.opt()], outs=[src_tbl.opt()])

            # ---- bulk-load all window metadata into SBUF
            sidxall = cp.tile([128, nW, NSUB * 32], i16)
            nc.sync.dma_start(out=sidxall[:], in_=sidx_e[:])
            dlocall = cp.tile([128, nW, K], f32)
            nc.sync.dma_start(out=dlocall[:], in_=dloc_e[:])
            nidall = cp.tile([128, nW, 1], i32)
            nc.sync.dma_start(out=nidall[:], in_=nid_e[:])

            # ---- main loop (2-stage software pipeline)
            def stage1(w):
                """loads + gathers + qd + score chain for window w"""
                st = {}
                sidx = sidxall[:, w, :]
                dloc = dlocall[:, w, :]
                dstrow = sp.tile([128, ROWD], f16, tag="dr")
                nc.gpsimd.indirect_dma_start(
                    out=dstrow[:], out_offset=None, in_=dst_tbl[:],
                    in_offset=bass.IndirectOffsetOnAxis(
                        ap=nidall[:, w, :], axis=0))
                # qb[p, j] = q_dst of window node j (broadcast to all p)
                qT_ps = pp2.tile([1, 128], f16, space="PSUM", tag="qt")
                nc.tensor.transpose(out=qT_ps[:], in_=dstrow[:, 65:66],
                                    identity=ident16[:])
                qrow = sp.tile([1, 128], f16, tag="qr")
                nc.vector.tensor_copy(qrow[:], qT_ps[:])
                qb_ps = pp2.tile([128, 128], f32, space="PSUM", tag="qb")
                nc.tensor.matmul(out=qb_ps[:], lhsT=ones_row[:], rhs=qrow[:],
                                 start=True, stop=True)
                qb = sp.tile([128, 128], f16, tag="qbs")
                nc.vector.tensor_copy(qb[:], qb_ps[:])

                # edge payload gathers (4 subtables x 512 edges)
                pay = sp.tile([128, K, ROWS], f16, tag="pay", bufs=3)
                if ablate != "compute_only":
                    for s in range(NSUB):
                        nc.gpsimd.dma_gather(
                            pay[:, s * 4:(s + 1) * 4, :],
                            src_tbl[s * SUBR:(s + 1) * SUBR, :],
                            sidx[:, s * 32:(s + 1) * 32],
                            GCAP, GCAP, ROWS, queue_num=s)
                else:
                    nc.vector.memset(pay[:], 1.0)

                # qd[p,k] = q_dst of edge (p,k)'s destination
                qd = sp.tile([128, K], f16, tag="qd")
                for k in range(K):
                    scr = sp.tile([128, 128], f16, tag="scr", bufs=4)
                    nc.vector.scalar_tensor_tensor(
                        out=scr[:], in0=iota16[:], scalar=dloc[:, k:k + 1],
                        in1=qb[:], op0=AL.is_equal, op1=AL.mult,
                        accum_out=qd[:, k:k + 1])
                # w = exp(tanh(qd - qs)), all tiles at once
                dsub = sp.tile([128, K], f16, tag="dsu")
                nc.vector.tensor_tensor(dsub[:], qd[:], pay[:, :, 65],
                                        op=AL.subtract)
                th = sp.tile([128, K], f16, tag="th")
                nc.scalar.activation(out=th[:], in_=dsub[:], func=AF.Tanh)
                wgt = sp.tile([128, K], f32, tag="wg")
                nc.scalar.activation(out=wgt[:], in_=th[:], func=AF.Exp)
                st.update(pay=pay, dloc=dloc, wgt=wgt, dstrow=dstrow)
                return st

            def stage2(w, st):
                """weighted scatter-matmul + epilogue for window w"""
                pay, dloc, wgt, dstrow = (st["pay"], st["dloc"], st["wgt"],
                                          st["dstrow"])
                acc = pp2.tile([128, 65], f32, space="PSUM", tag="acc", bufs=3)
                for k in range(K):
                    Sw = sp.tile([128, 128], f16, tag="sw", bufs=6)
                    nc.vector.tensor_scalar(
                        out=Sw[:], in0=iota16[:], scalar1=dloc[:, k:k + 1],
                        scalar2=wgt[:, k:k + 1], op0=AL.is_equal, op1=AL.mult)
                    nc.tensor.matmul(out=acc[:], lhsT=Sw[:],
                                     rhs=pay[:, k, 0:65],
                                     start=(k == 0), stop=(k == K - 1))

                # epilogue: out = elu(p_dst - swp/sw) * (sw != 0)
                # every real edge has w >= e^-1, so sw >= 0.367 or sw == 0
                den = sp.tile([128, 1], f32, tag="den")
                nc.vector.tensor_scalar(
                    out=den[:], in0=acc[:, 64:65], scalar1=0.3, scalar2=None,
                    op0=AL.max)
                rec = sp.tile([128, 1], f32, tag="rec")
                nc.vector.reciprocal(rec[:], den[:])
                nzm = sp.tile([128, 1], f32, tag="nzm")
                nc.vector.tensor_scalar(
                    out=nzm[:], in0=acc[:, 64:65], scalar1=3.0, scalar2=1.0,
                    op0=AL.mult, op1=AL.min)
                mean = sp.tile([128, D], f32, tag="mean")
                nc.scalar.activation(out=mean[:], in_=acc[:, 0:64],
                                     func=AF.Copy, scale=rec[:])
                pd = sp.tile([128, D], f32, tag="pd")
                nc.vector.tensor_copy(pd[:], dstrow[:, 0:64])
                diff = sp.tile([128, D], f32, tag="diff")
                nc.vector.tensor_tensor(diff[:], pd[:], mean[:],
                                        op=AL.subtract)
                dm = sp.tile([128, D], f32, tag="dm")
                nc.vector.tensor_scalar(
                    out=dm[:], in0=diff[:], scalar1=nzm[:], scalar2=None,
                    op0=AL.mult)
                nr = sp.tile([128, D], f32, tag="nr2")
                nc.scalar.activation(out=nr[:], in_=dm[:], func=AF.Relu,
                                     scale=-1.0)
                ex = sp.tile([128, D], f32, tag="ex")
                nc.scalar.activation(out=ex[:], in_=nr[:], func=AF.Exp,
                                     scale=-1.0)
                pos = sp.tile([128, D], f32, tag="pos")
                nc.scalar.activation(out=pos[:], in_=dm[:], func=AF.Relu)
                res = sp.tile([128, D], f32, tag="res")
                nc.vector.scalar_tensor_tensor(
                    out=res[:], in0=ex[:], scalar=-1.0, in1=pos[:],
                    op0=AL.add, op1=AL.add)
                nc.sync.dma_start(out=res_e[w * 128:(w + 1) * 128, :],
                                  in_=res[:])

            def gather_only_body(w):
                sidx = sidxall[:, w, :]
                dstrow = sp.tile([128, ROWD], f16, tag="dr")
                nc.gpsimd.indirect_dma_start(
                    out=dstrow[:], out_offset=None, in_=dst_tbl[:],
                    in_offset=bass.IndirectOffsetOnAxis(
                        ap=nidall[:, w, :], axis=0))
                pay = sp.tile([128, K, ROWS], f16, tag="pay", bufs=3)
                for s in range(NSUB):
                    nc.gpsimd.dma_gather(
                        pay[:, s * 4:(s + 1) * 4, :],
                        src_tbl[s * SUBR:(s + 1) * SUBR, :],
                        sidx[:, s * 32:(s + 1) * 32],
                        GCAP, GCAP, ROWS, queue_num=s)
                gres = sp.tile([128, D], f32, tag="res")
                nc.vector.tensor_copy(gres[:], pay[:, 0, 0:D])
                nc.sync.dma_start(out=res_e[w * 128:(w + 1) * 128, :],
                                  in_=gres[:])

            if ablate == "prologue_repeat":
                gres = sp.tile([128, D], f32, tag="res")
                nc.vector.tensor_copy(gres[:], iota16[:, 0:D])
                nc.sync.dma_start(out=res_e[0:128, :], in_=gres[:])
                rep_ctx = None
            else:
                rep_ctx = (tc.For_i(0, main_repeat, 1)
                           if main_repeat > 1 else None)
            if rep_ctx is not None:
                rep_ctx.__enter__()
            if ablate == "prologue_repeat":
                pass
            elif ablate == "gather_only":
                for w in range(nW):
                    gather_only_body(w)
            else:
                pend = stage1(0)
                for w in range(nW):
                    nxt = stage1(w + 1) if w + 1 < nW else None
                    stage2(w, pend)
                    pend = nxt
            if rep_ctx is not None:
                rep_ctx.__exit__(None, None, None)
            pp2.release()
    nc.compile()
    return nc


_CACHE = {}


def _get_program(nW, main_repeat, ablate=""):
    key = (nW, main_repeat, ablate)
    if key not in _CACHE:
        _CACHE[key] = _build_program(nW, main_repeat, ablate)
    return _CACHE[key]


def kernel(h_src, h_dst, W_fc, w_attn, src, dst, _main_repeat=MAIN_REPEAT,
           _return_walls=False, _ablate=""):
    from concourse.bass_utils import run_bass_kernel_spmd

    h_src = np.ascontiguousarray(np.asarray(h_src, np.float32))
    h_dst = np.ascontiguousarray(np.asarray(h_dst, np.float32))
    W_fc = np.ascontiguousarray(np.asarray(W_fc, np.float32))
    w_attn = np.ascontiguousarray(np.asarray(w_attn, np.float32)).reshape(D, 1)
    cores, nW = _prep(src, dst)

    hp = np.zeros((NPAD, D), np.float32); hp[:N] = h_src
    hq = np.zeros((N + DROWS, D), np.float32); hq[:N] = h_dst

    in_maps = []
    for c, core in enumerate(cores):
        n_lo = core["n_lo"]
        in_maps.append({
            "hs": np.ascontiguousarray(hp[c * SHARD:(c + 1) * SHARD].T),
            "hd": np.ascontiguousarray(hq[n_lo:n_lo + DROWS].T),
            "wfc": W_fc,
            "wat": w_attn,
            "sidx": np.ascontiguousarray(core["sidx"].transpose(1, 0, 2)),
            "dloc": np.ascontiguousarray(core["dloc"].transpose(1, 0, 2)),
            "nid": np.ascontiguousarray(
                core["slot_ids"].T[:, :, None]),
            })
    nc = _get_program(nW, _main_repeat, _ablate)
    import time
    walls = []
    t0 = time.time()
    res = run_bass_kernel_spmd(nc, in_maps, list(range(NC)))
    walls.append(time.time() - t0)

    out = np.zeros((N, D), np.float32)
    for c, core in enumerate(cores):
        r = res.results[c]["res"].reshape(nW, 128, D)
        base, nn = core["base"], core["nn"]
        for w in range(nW):
            if nn[w] > 0:
                out[base[w]:base[w] + nn[w]] = r[w, :nn[w]]
    if _return_walls:
        return out, walls
    return out


if __name__ == "__main__":
    pass


# revision 6
# speedup vs baseline: 1.3325x; 1.3325x over previous
"""Trainium2 Bass kernel for nn_DiffAttention (GNN message passing).

Math (per edge i: src s_i -> dst n, dst sorted):
  d_i = (h_dst[n] - h_src[s_i]) @ W_fc.T ;  e_i = tanh(d_i @ w_attn)
  alpha = segment_softmax(e, dst);  out[n] = elu(sum_i alpha_i d_i)
Since e in [-1,1], softmax needs no max-subtraction:
  out[n] = elu(p_dst[n] - (sum_i w_i p_src[s_i]) / (sum_i w_i)),
  w_i = exp(tanh(q_dst[n] - q_src[s_i])), p = h @ W_fc.T, q = p @ w_attn.

v2 device strategy (8 cores, SPMD, edge-parallel by dst range):
  - src table [NPAD, 128] f16 rows [p_src(0:64) | 1 | q_src | pad], built
    sharded on PE then AllGather; dst table [NPAD, 66] f16 rows
    [p_dst | _ | q_dst].
  - per window (<=128 consecutive dst nodes, 2048 edge slots grouped as
    4 subtables x 512): 4 dma_gather ops (int16 subtable-local indices)
    fetch all edge rows; window dst rows via one [128,1] indirect gather.
  - qb[p, j] = q_dst(window node j) broadcast via K=1 matmul.
  - per 128-edge tile: one fused DVE op gives qd (accum of onehot*qb);
    batched tanh/exp on ACT for all 16 tiles at once; one fused DVE op
    builds Sw = onehot*w (f16); PE accumulates [sum w*p | sum w] in PSUM.
  - epilogue per window: elu(p_dst - swp/sw) with zero-edge masking.
Host does only index prep (windowing, grouping by src subtable, int16
wrapping, f16 casts) and reassembles per-window outputs into node order.
"""
import sys
sys.path.insert(0, "/opt/trn_rl_repo")
import numpy as np

N = 100000
D = 64
NC = 8
K = 16            # 128-edge tiles per window (4 groups x 4 tiles)
NSUB = 4          # src-node subtables (int16 index range)
SUBR = 25088      # rows per subtable
GCAP = 512        # edge slots per (window, subtable) group
WE = NSUB * GCAP  # 2048 edge slots per window
WIN_NODES = 128
SHARD = 12544     # 98*128 rows built per core
DROWS = 12800     # local dst-table rows (core's window node range, padded)
NPAD = NC * SHARD # 100352 = 4*25088
DUMMY = N         # all-zero row (h padded with zeros)
ROWS = 128        # src table row width (f16): [p(0:64) | 1 | q | pad]
ROWD = 66         # dst table row width (f16): [p(0:64) | _ | q]
MAIN_REPEAT = 1   # test.py overrides for timing


# ---------------------------------------------------------------- host prep
def _partition_edges(dst):
    E = dst.shape[0]
    bounds, e_prev, n_prev = [], 0, 0
    for c in range(1, NC):
        s = (E * c) // NC
        while 0 < s < E and dst[s] == dst[s - 1]:
            s += 1
        node_split = int(dst[s]) if s < E else N
        bounds.append((e_prev, s, n_prev, node_split))
        e_prev, n_prev = s, node_split
    bounds.append((e_prev, E, n_prev, N))
    return bounds


def _build_windows(src, dst, e_lo, e_hi, n_lo, n_hi):
    n_total = n_hi - n_lo
    dl = dst[e_lo:e_hi] - n_lo
    sub = src[e_lo:e_hi] // SUBR
    # per-subtable cumulative edge counts by node
    cums = np.zeros((NSUB, n_total + 1), np.int64)
    for s in range(NSUB):
        cums[s, 1:] = np.cumsum(np.bincount(
            dl[sub == s], minlength=n_total))
    counts = np.bincount(dl, minlength=n_total)
    starts = np.concatenate([[0], np.cumsum(counts)])
    sidxs, dlocs, bases, nns = [], [], [], []
    n = 0
    while n < n_total:
        n_end = min(n + WIN_NODES, n_total)
        while n_end > n and np.any(cums[:, n_end] - cums[:, n] > GCAP):
            n_end -= 1
        assert n_end > n, "node degree exceeds per-subtable window cap"
        e0, e1 = int(starts[n]), int(starts[n_end])
        esub = sub[e0:e1]
        edl = dl[e0:e1] - n
        esrc = src[e_lo + e0:e_lo + e1]
        sidx16 = np.zeros((128, NSUB * 32), np.int16)
        dloc = np.full((128, K), -1.0, np.float32)
        for s in range(NSUB):
            sel = esub == s
            c = int(sel.sum())
            arr = np.zeros(GCAP, np.int16)
            arr[:c] = (esrc[sel] - s * SUBR).astype(np.int16)
            wrapped = arr.reshape(32, 16).T                 # [16, 32]
            sidx16[:, s * 32:(s + 1) * 32] = np.tile(wrapped, (8, 1))
            i = np.arange(c)
            dloc[i % 128, s * 4 + i // 128] = edl[sel].astype(np.float32)
        sidxs.append(sidx16); dlocs.append(dloc)
        bases.append(n_lo + n); nns.append(n_end - n)
        n = n_end
    return np.stack(sidxs), np.stack(dlocs), np.array(bases), np.array(nns)


def _prep(src, dst):
    src = np.asarray(src, np.int64)
    dst = np.asarray(dst, np.int64)
    if np.any(np.diff(dst) < 0):  # tolerate unsorted edges
        order = np.argsort(dst, kind="stable")
        src, dst = src[order], dst[order]
    bounds = _partition_edges(dst)
    per_core = [_build_windows(src, dst, *b) for b in bounds]
    nW = max(p[0].shape[0] for p in per_core)
    cores = []
    for (e_lo, e_hi, n_lo, n_hi), (si, dl, base, nn) in zip(bounds, per_core):
        assert n_hi - n_lo <= DROWS - 128, "core node range exceeds DROWS"
        pad = nW - si.shape[0]
        if pad:
            si = np.concatenate([si, np.zeros((pad, 128, NSUB * 32), np.int16)])
            dl = np.concatenate([dl, np.full((pad, 128, K), -1.0, np.float32)])
            base = np.concatenate([base, np.full(pad, N, np.int64)])
            nn = np.concatenate([nn, np.zeros(pad, np.int64)])
        slot = (base[:, None] - n_lo) + np.arange(WIN_NODES)[None, :]
        slot = np.where(np.arange(WIN_NODES)[None, :] < nn[:, None], slot, 0)
        slot = np.clip(slot, 0, DROWS - 1)
        cores.append(dict(sidx=si, dloc=dl, base=base, nn=nn, n_lo=n_lo,
                          slot_ids=slot.astype(np.int32)))
    return cores, nW


# ---------------------------------------------------------------- device
def _build_program(nW, main_repeat, ablate=""):
    from concourse import bass, bacc, mybir, tile
    f32, f16 = mybir.dt.float32, mybir.dt.float16
    i16, i32 = mybir.dt.int16, mybir.dt.int32

    nc = bacc.Bacc("TRN2", target_bir_lowering=False, debug=False,
                   num_devices=NC, num_swdge_queues=4)
    hs_e = nc.dram_tensor("hs", [D, SHARD], f32, kind="ExternalInput")
    hd_e = nc.dram_tensor("hd", [D, DROWS], f32, kind="ExternalInput")
    wfc_e = nc.dram_tensor("wfc", [D, D], f32, kind="ExternalInput")
    wat_e = nc.dram_tensor("wat", [D, 1], f32, kind="ExternalInput")
    sidx_e = nc.dram_tensor("sidx", [128, nW, NSUB * 32], i16,
                            kind="ExternalInput")
    dloc_e = nc.dram_tensor("dloc", [128, nW, K], f32, kind="ExternalInput")
    nid_e = nc.dram_tensor("nid", [128, nW, 1], i32, kind="ExternalInput")
    res_e = nc.dram_tensor("res", [nW * 128, D], f32, kind="ExternalOutput")

    AL = mybir.AluOpType
    AF = mybir.ActivationFunctionType

    with tile.TileContext(nc) as tc:
        with tc.tile_pool(name="c", bufs=1) as cp, \
             tc.tile_pool(name="sb", bufs=3) as sp, \
             tc.tile_pool(name="dr", bufs=1, space="DRAM") as dp:
            pp = tc.alloc_tile_pool(name="psb", bufs=1, space="PSUM")
            # ---- constants
            ident_d = nc.inline_tensor(np.eye(128, dtype=np.float32),
                                       name="ident_c")
            iota16_d = nc.inline_tensor(
                np.tile(np.arange(128, dtype=np.float16), (128, 1)),
                name="iota16_c")
            ident = cp.tile([128, 128], f32)
            nc.sync.dma_start(out=ident[:], in_=ident_d[:])
            iota16 = cp.tile([128, 128], f16)
            nc.sync.dma_start(out=iota16[:], in_=iota16_d[:])
            ident16_d = nc.inline_tensor(np.eye(128, dtype=np.float16),
                                         name="ident16_c")
            ident16 = cp.tile([128, 128], f16)
            nc.sync.dma_start(out=ident16[:], in_=ident16_d[:])
            ones_row = cp.tile([1, 128], f16)
            nc.vector.memset(ones_row[:], 1.0)
            ones_col = cp.tile([128, 1], f32)
            nc.vector.memset(ones_col[:], 1.0)

            # ---- weight prep: rhsb [64, 66] = [W.T | 0 | W.T @ w_attn]
            wfc = cp.tile([D, D], f32)
            nc.sync.dma_start(out=wfc[:], in_=wfc_e[:])
            wat = cp.tile([D, 1], f32)
            nc.sync.dma_start(out=wat[:], in_=wat_e[:])
            wt_ps = pp.tile([D, D], f32, space="PSUM")
            nc.tensor.transpose(out=wt_ps[:], in_=wfc[:], identity=ident[:D, :D])
            v_ps = pp.tile([D, 1], f32, space="PSUM")
            nc.tensor.matmul(out=v_ps[:], lhsT=wfc[:], rhs=wat[:],
                             start=True, stop=True)
            rhsb = cp.tile([D, ROWD], f32)
            nc.vector.memset(rhsb[:], 0.0)
            nc.vector.tensor_copy(rhsb[:, 0:64], wt_ps[:])
            nc.vector.tensor_copy(rhsb[:, 65:66], v_ps[:])

            # ---- table build: src rows from this core's shard,
            # dst rows local to this core's window node range
            bp = tc.alloc_tile_pool(name="bld", bufs=1)
            src_sh = dp.tile([SHARD, ROWS], f16)
            dst_tbl = dp.tile([DROWS, ROWD], f16)
            pro_ctx = (tc.For_i(0, main_repeat, 1)
                       if ablate == "prologue_repeat" and main_repeat > 1
                       else None)
            if pro_ctx is not None:
                pro_ctx.__enter__()
            hsT = bp.tile([D, SHARD], f32)
            nc.sync.dma_start(out=hsT[:], in_=hs_e[:])
            hdT = bp.tile([D, DROWS], f32)
            nc.sync.dma_start(out=hdT[:], in_=hd_e[:])
            for j in range(SHARD // 128):
                r0 = j * 128
                ps_s = pp.tile([128, ROWD], f32, space="PSUM", tag="bp", bufs=3)
                nc.tensor.matmul(out=ps_s[:], lhsT=hsT[:, r0:r0 + 128],
                                 rhs=rhsb[:], start=True, stop=True)
                tbs = sp.tile([128, ROWS], f16, tag="bo")
                nc.vector.memset(tbs[:], 0.0)
                nc.vector.tensor_copy(tbs[:, 0:ROWD], ps_s[:])
                nc.vector.memset(tbs[:, 64:65], 1.0)
                if j % 2 == 0:
                    nc.sync.dma_start(out=src_sh[r0:r0 + 128, :], in_=tbs[:])
                else:
                    nc.scalar.dma_start(out=src_sh[r0:r0 + 128, :], in_=tbs[:])
            for j in range(DROWS // 128):
                r0 = j * 128
                ps_d = pp.tile([128, ROWD], f32, space="PSUM", tag="bp2", bufs=3)
                nc.tensor.matmul(out=ps_d[:], lhsT=hdT[:, r0:r0 + 128],
                                 rhs=rhsb[:], start=True, stop=True)
                tbd = sp.tile([128, ROWD], f16, tag="bo2")
                nc.vector.tensor_copy(tbd[:], ps_d[:])
                if j % 2 == 0:
                    nc.sync.dma_start(out=dst_tbl[r0:r0 + 128, :], in_=tbd[:])
                else:
                    nc.scalar.dma_start(out=dst_tbl[r0:r0 + 128, :], in_=tbd[:])
            if pro_ctx is not None:
                pro_ctx.__exit__(None, None, None)
                pro_ctx = None
            bp.release()

            pp.release()
            pp2 = tc.alloc_tile_pool(name="psm", bufs=2, space="PSUM")

            # ---- all-gather the tables

            # ---- bulk-load all window metadata into SBUF
            sidxall = cp.tile([128, nW, NSUB * 32], i16)
            nc.sync.dma_start(out=sidxall[:], in_=sidx_e[:])
            dlocall = cp.tile([128, nW, K], f32)
            nc.sync.dma_start(out=dlocall[:], in_=dloc_e[:])
            nidall = cp.tile([128, nW, 1], i32)
            nc.sync.dma_start(out=nidall[:], in_=nid_e[:])

            src_tbl = dp.tile([NPAD, ROWS], f16)
            nc.gpsimd.collective_compute(
                "AllGather", mybir.AluOpType.bypass,
                replica_groups=[list(range(NC))],
                ins=[src_sh.opt()], outs=[src_tbl.opt()])

            # ---- main loop (2-stage software pipeline)
            def stage1(w):
                """loads + gathers + qd + score chain for window w"""
                st = {}
                sidx = sidxall[:, w, :]
                dloc = dlocall[:, w, :]
                dstrow = sp.tile([128, ROWD], f16, tag="dr")
                nc.gpsimd.indirect_dma_start(
                    out=dstrow[:], out_offset=None, in_=dst_tbl[:],
                    in_offset=bass.IndirectOffsetOnAxis(
                        ap=nidall[:, w, :], axis=0))
                # qb[p, j] = q_dst of window node j (broadcast to all p)
                qT_ps = pp2.tile([1, 128], f16, space="PSUM", tag="qt")
                nc.tensor.transpose(out=qT_ps[:], in_=dstrow[:, 65:66],
                                    identity=ident16[:])
                qrow = sp.tile([1, 128], f16, tag="qr")
                nc.vector.tensor_copy(qrow[:], qT_ps[:])
                qb_ps = pp2.tile([128, 128], f32, space="PSUM", tag="qb")
                nc.tensor.matmul(out=qb_ps[:], lhsT=ones_row[:], rhs=qrow[:],
                                 start=True, stop=True)
                qb = sp.tile([128, 128], f16, tag="qbs")
                nc.vector.tensor_copy(qb[:], qb_ps[:])

                # edge payload gathers (4 subtables x 512 edges)
                pay = sp.tile([128, K, ROWS], f16, tag="pay", bufs=3)
                if ablate != "compute_only":
                    for s in range(NSUB):
                        nc.gpsimd.dma_gather(
                            pay[:, s * 4:(s + 1) * 4, :],
                            src_tbl[s * SUBR:(s + 1) * SUBR, :],
                            sidx[:, s * 32:(s + 1) * 32],
                            GCAP, GCAP, ROWS, queue_num=s)
                else:
                    nc.vector.memset(pay[:], 1.0)

                # qd[p,k] = q_dst of edge (p,k)'s destination
                qd = sp.tile([128, K], f16, tag="qd")
                for k in range(K):
                    scr = sp.tile([128, 128], f16, tag="scr", bufs=4)
                    nc.vector.scalar_tensor_tensor(
                        out=scr[:], in0=iota16[:], scalar=dloc[:, k:k + 1],
                        in1=qb[:], op0=AL.is_equal, op1=AL.mult,
                        accum_out=qd[:, k:k + 1])
                # w = exp(tanh(qd - qs)), all tiles at once
                dsub = sp.tile([128, K], f16, tag="dsu")
                nc.vector.tensor_tensor(dsub[:], qd[:], pay[:, :, 65],
                                        op=AL.subtract)
                th = sp.tile([128, K], f16, tag="th")
                nc.scalar.activation(out=th[:], in_=dsub[:], func=AF.Tanh)
                wgt = sp.tile([128, K], f32, tag="wg")
                nc.scalar.activation(out=wgt[:], in_=th[:], func=AF.Exp)
                st.update(pay=pay, dloc=dloc, wgt=wgt, dstrow=dstrow)
                return st

            def stage2(w, st):
                """weighted scatter-matmul + epilogue for window w"""
                pay, dloc, wgt, dstrow = (st["pay"], st["dloc"], st["wgt"],
                                          st["dstrow"])
                acc = pp2.tile([128, 65], f32, space="PSUM", tag="acc", bufs=3)
                for k in range(K):
                    Sw = sp.tile([128, 128], f16, tag="sw", bufs=6)
                    nc.vector.tensor_scalar(
                        out=Sw[:], in0=iota16[:], scalar1=dloc[:, k:k + 1],
                        scalar2=wgt[:, k:k + 1], op0=AL.is_equal, op1=AL.mult)
                    nc.tensor.matmul(out=acc[:], lhsT=Sw[:],
                                     rhs=pay[:, k, 0:65],
                                     start=(k == 0), stop=(k == K - 1))

                # epilogue: out = elu(p_dst - swp/sw) * (sw != 0)
                # every real edge has w >= e^-1, so sw >= 0.367 or sw == 0
                den = sp.tile([128, 1], f32, tag="den")
                nc.vector.tensor_scalar(
                    out=den[:], in0=acc[:, 64:65], scalar1=0.3, scalar2=None,
                    op0=AL.max)
                rec = sp.tile([128, 1], f32, tag="rec")
                nc.vector.reciprocal(rec[:], den[:])
                nzm = sp.tile([128, 1], f32, tag="nzm")
                nc.vector.tensor_scalar(
                    out=nzm[:], in0=acc[:, 64:65], scalar1=3.0, scalar2=1.0,
                    op0=AL.mult, op1=AL.min)
                mean = sp.tile([128, D], f32, tag="mean")
                nc.scalar.activation(out=mean[:], in_=acc[:, 0:64],
                                     func=AF.Copy, scale=rec[:])
                pd = sp.tile([128, D], f32, tag="pd")
                nc.vector.tensor_copy(pd[:], dstrow[:, 0:64])
                diff = sp.tile([128, D], f32, tag="diff")
                nc.vector.tensor_tensor(diff[:], pd[:], mean[:],
                                        op=AL.subtract)
                dm = sp.tile([128, D], f32, tag="dm")
                nc.vector.tensor_scalar(
                    out=dm[:], in0=diff[:], scalar1=nzm[:], scalar2=None,
                    op0=AL.mult)
                nr = sp.tile([128, D], f32, tag="nr2")
                nc.scalar.activation(out=nr[:], in_=dm[:], func=AF.Relu,
                                     scale=-1.0)
                ex = sp.tile([128, D], f32, tag="ex")
                nc.scalar.activation(out=ex[:], in_=nr[:], func=AF.Exp,
                                     scale=-1.0)
                pos = sp.tile([128, D], f32, tag="pos")
                nc.scalar.activation(out=pos[:], in_=dm[:], func=AF.Relu)
                res = sp.tile([128, D], f32, tag="res")
                nc.vector.scalar_tensor_tensor(
                    out=res[:], in0=ex[:], scalar=-1.0, in1=pos[:],
                    op0=AL.add, op1=AL.add)
                nc.sync.dma_start(out=res_e[w * 128:(w + 1) * 128, :],
                                  in_=res[:])

            def gather_only_body(w):
                sidx = sidxall[:, w, :]
                dstrow = sp.tile([128, ROWD], f16, tag="dr")
                nc.gpsimd.indirect_dma_start(
                    out=dstrow[:], out_offset=None, in_=dst_tbl[:],
                    in_offset=bass.IndirectOffsetOnAxis(
                        ap=nidall[:, w, :], axis=0))
                pay = sp.tile([128, K, ROWS], f16, tag="pay", bufs=3)
                for s in range(NSUB):
                    nc.gpsimd.dma_gather(
                        pay[:, s * 4:(s + 1) * 4, :],
                        src_tbl[s * SUBR:(s + 1) * SUBR, :],
                        sidx[:, s * 32:(s + 1) * 32],
                        GCAP, GCAP, ROWS, queue_num=s)
                gres = sp.tile([128, D], f32, tag="res")
                nc.vector.tensor_copy(gres[:], pay[:, 0, 0:D])
                nc.sync.dma_start(out=res_e[w * 128:(w + 1) * 128, :],
                                  in_=gres[:])

            if ablate == "prologue_repeat":
                gres = sp.tile([128, D], f32, tag="res")
                nc.vector.tensor_copy(gres[:], iota16[:, 0:D])
                nc.sync.dma_start(out=res_e[0:128, :], in_=gres[:])
                rep_ctx = None
            else:
                rep_ctx = (tc.For_i(0, main_repeat, 1)
                           if main_repeat > 1 else None)
            if rep_ctx is not None:
                rep_ctx.__enter__()
            if ablate == "prologue_repeat":
                pass
            elif ablate == "gather_only":
                for w in range(nW):
                    gather_only_body(w)
            else:
                pend = stage1(0)
                for w in range(nW):
                    nxt = stage1(w + 1) if w + 1 < nW else None
                    stage2(w, pend)
                    pend = nxt
            if rep_ctx is not None:
                rep_ctx.__exit__(None, None, None)
            pp2.release()
    nc.compile()
    return nc


_CACHE = {}


def _get_program(nW, main_repeat, ablate=""):
    key = (nW, main_repeat, ablate)
    if key not in _CACHE:
        _CACHE[key] = _build_program(nW, main_repeat, ablate)
    return _CACHE[key]


def kernel(h_src, h_dst, W_fc, w_attn, src, dst, _main_repeat=MAIN_REPEAT,
           _return_walls=False, _ablate=""):
    from concourse.bass_utils import run_bass_kernel_spmd

    h_src = np.ascontiguousarray(np.asarray(h_src, np.float32))
    h_dst = np.ascontiguousarray(np.asarray(h_dst, np.float32))
    W_fc = np.ascontiguousarray(np.asarray(W_fc, np.float32))
    w_attn = np.ascontiguousarray(np.asarray(w_attn, np.float32)).reshape(D, 1)
    cores, nW = _prep(src, dst)

    hp = np.zeros((NPAD, D), np.float32); hp[:N] = h_src
    hq = np.zeros((N + DROWS, D), np.float32); hq[:N] = h_dst

    in_maps = []
    for c, core in enumerate(cores):
        n_lo = core["n_lo"]
        in_maps.append({
            "hs": np.ascontiguousarray(hp[c * SHARD:(c + 1) * SHARD].T),
            "hd": np.ascontiguousarray(hq[n_lo:n_lo + DROWS].T),
            "wfc": W_fc,
            "wat": w_attn,
            "sidx": np.ascontiguousarray(core["sidx"].transpose(1, 0, 2)),
            "dloc": np.ascontiguousarray(core["dloc"].transpose(1, 0, 2)),
            "nid": np.ascontiguousarray(
                core["slot_ids"].T[:, :, None]),
            })
    nc = _get_program(nW, _main_repeat, _ablate)
    import time
    walls = []
    t0 = time.time()
    res = run_bass_kernel_spmd(nc, in_maps, list(range(NC)))
    walls.append(time.time() - t0)

    out = np.zeros((N, D), np.float32)
    for c, core in enumerate(cores):
        r = res.results[c]["res"].reshape(nW, 128, D)
        base, nn = core["base"], core["nn"]
        for w in range(nW):
            if nn[w] > 0:
                out[base[w]:base[w] + nn[w]] = r[w, :nn[w]]
    if _return_walls:
        return out, walls
    return out


if __name__ == "__main__":
    pass
